# revision 2
# baseline (speedup 1.0000x reference)
"""Trainium2 Bass kernel v2: ExponentialConcordanceLoss via antisymmetric
pair-orientation (tournament) halving.

loss = S / T,  S = sum_{a,b: d_a<d_b} e_a exp(p_b - p_a),  T = #pairs.

Pairs split by 128-row tile (64 tiles):
 - cross-tile pairs (t,u) in tournament R ((u-t) mod 64 in 1..31, or ==32
   with t<32): both orientations fold via
     m_ab c_a w_b + m_ba c_b w_a = c_b w_a + m_ab (c_a w_b - c_b w_a)
   so only canonical masks are generated (HALF the comparisons).  The
   closed term sum_R W_t C_u (and pair count) is injected into the PSUM
   accumulators by a matmul against a chunk-expanded tournament matrix
   Q64x [64 x 1024].
 - same-tile pairs: direct both-orientation masks, batched as one
   [128, 1024] compare in (slot, tile) column order so the 16x broadcast
   APs stay 2x-mode eligible; consumed by matmuls with (tile, slot)
   traversal back into the main accumulators.

Sharding: core k owns j = 128u + 8s + k (u=tile/chunk, s=slot), local
order jl = 16u + s.  Tile t's cross mask covers a 512 (496 for t>=32)
wide window at column 16(t+1) of the chunk-doubled dj row [1536]; PSUM
accumulates in doubled column space, the fold adds the aliases.

Masks: Vector tensor_scalar is_gt {0,1} and Scalar ACT Tanh(BIG*(dj-d_i))
in {-1,+1} (exactly saturated; 0 on bf16 ties).  Tanh tiles use 0.5x
stationaries; their per-window deficit is repaired by one matmul against
a host-built coverage matrix V64x [64 x 1024] (0.5 where tile t is
Tanh-assigned and covers jl's chunk).  A single ACT function set
(exp_and_others: Exp+Tanh+Copy) means exactly one table load.

Stationaries per tile: ce4[:, :, t] = [c_a, e_a, -w_a, -1] (M=4), fold
rows per group: [w_row, ones, c_row, e_row]; all small row vectors are
built in [128, 8] column layout and DMA-transposed into the fold rows.
Both comparison sides use bf16-rounded durations so m_ab + m_ba = 1
holds except exact ties (handled by the tanh-tie midpoint).
"""

import numpy as np
import ml_dtypes

N = 8192
NCORES = 8
P = 128
NT = 64          # i-tiles / j-chunks
JL = 1024        # j per core
DW = 1536        # doubled-window column space
NG = 4           # PE column groups
BIG = float(2 ** 30)

_BF16 = ml_dtypes.bfloat16

# ACT-engine (tanh) mask tiles; rest on Vector.
def _is_act_tile(t):
    return t % 3 == 1 or t in (12, 32, 48)


def _win(t):
    return 16 * (t + 1), (512 if t < 32 else 496)


_cached = None


def _build():
    from concourse import bacc, tile, mybir

    dt = mybir.dt
    Alu = mybir.AluOpType
    Act = mybir.ActivationFunctionType
    AX = mybir.AxisListType

    nc = bacc.Bacc("TRN2", target_bir_lowering=False, debug=False,
                   num_devices=NCORES)

    dj_d = nc.dram_tensor("dj", [P, JL], dt.bfloat16, kind="ExternalInput").ap()
    smalls_d = nc.dram_tensor("smalls", [P, 2 * NT + 8], dt.bfloat16,
                              kind="ExternalInput").ap()
    smallf_d = nc.dram_tensor("smallf", [P, 2 * NT + 8], dt.float32,
                              kind="ExternalInput").ap()
    ej_d = nc.dram_tensor("ej", [1, JL], dt.float32, kind="ExternalInput").ap()
    q64x_d = nc.dram_tensor("q64x", [NT, NT], dt.bfloat16, kind="ExternalInput").ap()
    v64x_d = nc.dram_tensor("v64x", [NT, NT], dt.bfloat16, kind="ExternalInput").ap()
    fsel_d = nc.dram_tensor("fsel", [P, 2], dt.float32, kind="ExternalInput").ap()
    out_d = nc.dram_tensor("out", [1, 2], dt.float32, kind="ExternalOutput").ap()

    with tile.TileContext(nc) as tc:
        with (
            tc.tile_pool(name="cpool", bufs=1) as cpool,
            tc.tile_pool(name="mpool", bufs=24) as mpool,
            tc.tile_pool(name="pspool", bufs=1, space="PSUM") as pspool,
        ):
            # ---- input DMAs; small/critical tensors first per queue
            dj_sb = cpool.tile([P, DW], dt.bfloat16)
            smalls = cpool.tile([P, 2 * NT + 8], dt.bfloat16)
            dcol_bf = smalls[:, 0:NT]
            ecol = smalls[:, NT:2 * NT]
            e8 = smalls[:, 2 * NT:2 * NT + 8]
            smallf = cpool.tile([P, 2 * NT + 8], dt.float32)
            pcol = smallf[:, 0:NT]
            pj8 = smallf[:, NT:NT + 8]
            dcol = smallf[:, NT + 8:2 * NT + 8]
            dj_su = cpool.tile([P, JL], dt.bfloat16)
            q64x = cpool.tile([NT, NT], dt.bfloat16)
            v64x = cpool.tile([NT, NT], dt.bfloat16)
            fsel = cpool.tile([P, 2], dt.float32)
            # sync queue: dj first half (V mask gate), then doubling copy
            nc.sync.dma_start(dj_sb[:, 0:512], dj_d[:, 0:512])
            nc.sync.dma_start(dj_sb[:, JL:DW], dj_sb[:, 0:512])
            nc.sync.dma_start(fsel[:], fsel_d[:])
            # gpsimd queue: dj second half
            nc.gpsimd.dma_start(dj_sb[:, 512:JL], dj_d[:, 512:JL])
            nc.gpsimd.dma_start(v64x[:], v64x_d[:])
            # scalar queue: the combined smalls first (S+V staging gate)
            nc.scalar.dma_start(smalls[:], smalls_d[:])
            nc.scalar.dma_start(smallf[:], smallf_d[:])
            nc.scalar.dma_start(q64x[:], q64x_d[:])

            # fold128 rows 32g+[0..3] = [w_row, 1, c_row, e_row]; built in
            # row4x [1, 4096] then partition-scattered (32-aligned dst).
            fold128 = cpool.tile([P, JL], dt.float32)
            nc.gpsimd.memset(fold128[:], 0.0)
            row4x = cpool.tile([1, 4 * JL], dt.float32)
            nc.gpsimd.memset(row4x[0:1, JL:2 * JL], 1.0)
            nc.sync.dma_start(row4x[0:1, 3 * JL:4 * JL], ej_d[:])

            zstat = cpool.tile([P, P], dt.bfloat16)
            nc.gpsimd.memset(zstat[:], 0.0)
            onesKf = cpool.tile([P, 1], dt.float32)
            nc.gpsimd.memset(onesKf[:], 1.0)
            onesKb = cpool.tile([P, 1], dt.bfloat16)
            nc.gpsimd.memset(onesKb[:], 1.0)
            # A-term and deficit stationaries [64, 4]
            st64a = cpool.tile([NT, 4], dt.bfloat16)
            nc.gpsimd.memset(st64a[:], 0.0)
            nc.gpsimd.memset(st64a[:, 3:4], float(P))
            st64d = cpool.tile([NT, 4], dt.bfloat16)
            nc.gpsimd.memset(st64d[:], 0.0)
            nc.gpsimd.memset(st64d[:, 3:4], -float(P))

            # ---- Scalar staging (one ACT table: Exp/Tanh/Copy)
            expnp = cpool.tile([P, NT], dt.float32)
            nc.scalar.activation(expnp[:], pcol[:], Act.Exp, scale=-1.0)
            expp = cpool.tile([P, NT], dt.float32)
            nc.scalar.activation(expp[:], pcol[:], Act.Exp)
            dbig = cpool.tile([P, NT], dt.float32)
            nc.scalar.activation(dbig[:], dcol_bf[:], Act.Copy, scale=-BIG)
            exp8 = cpool.tile([P, 8], dt.float32)
            nc.scalar.activation(exp8[:], pj8[:], Act.Exp)
            enp8 = cpool.tile([P, 8], dt.float32)
            nc.scalar.activation(enp8[:], pj8[:], Act.Exp, scale=-1.0)

            # ---- Vector staging tiles (ops emitted inside the mask loop
            # so Vector starts masks the moment dj/dcol land)
            ce4 = cpool.tile([P, 4, NT], dt.bfloat16)
            ccol = cpool.tile([P, NT], dt.float32)
            ceh4 = cpool.tile([P, 4, NT], dt.bfloat16)
            c8 = cpool.tile([P, 8], dt.float32)

            def emit_vstaging():
                nc.vector.tensor_copy(
                    dj_su[:].rearrange("p (s u) -> p s u", s=16, u=NT),
                    dj_sb[:, 0:JL].rearrange("p (u s) -> p s u",
                                             u=NT, s=16))
                nc.vector.tensor_mul(ccol[:], expnp[:], ecol[:])
                nc.vector.tensor_copy(ce4[:, 0, :], ccol[:])
                nc.vector.tensor_copy(ce4[:, 1, :], ecol[:])
                nc.vector.tensor_scalar(ce4[:, 2, :], expp[:], -1.0, None,
                                        Alu.mult)
                nc.vector.memset(ce4[:, 3, :], -1.0)
                nc.vector.tensor_scalar(
                    ceh4[:].rearrange("p a b -> p (a b)"),
                    ce4[:].rearrange("p a b -> p (a b)"), 0.5, None, Alu.mult)
                nc.vector.tensor_mul(c8[:], enp8[:], e8[:])
                # row transposes [128,8] -> [1,1024] (col = 8p + r)
                nc.sync.dma_start(
                    row4x[0:1, 0:JL].rearrange("o (p r) -> o p r", p=P, r=8),
                    exp8[:])
                nc.gpsimd.dma_start(
                    row4x[0:1, 2 * JL:3 * JL].rearrange("o (p r) -> o p r",
                                                        p=P, r=8),
                    c8[:])
                # fold row scatter
                for g in range(NG):
                    nc.sync.dma_start(fold128[32 * g:32 * g + 4, :], row4x[:])

            # ---- PSUM zero fill + accumulation group start
            ps = pspool.tile([P, DW], dt.float32, name="ps")
            for c in range(0, DW, 512):
                nc.tensor.matmul(ps[:, c:c + 512], zstat[:],
                                 dj_sb[:, c:c + 512],
                                 start=True, stop=False, skip_group_check=True)

            # ---- mask loop
            def emit_diag():
                # same-tile pairs in (slot, tile) order: broadcast APs keep
                # stride-1 last dim -> 2x DVE mode
                def bcsu(ap):
                    return ap.unsqueeze(1).broadcast_to([P, 16, NT])

                su = "p (s u) -> p s u"
                diagm = cpool.tile([P, JL], dt.bfloat16)
                nc.vector.tensor_tensor(
                    diagm[:].rearrange(su, s=16, u=NT),
                    dj_su[:].rearrange(su, s=16, u=NT),
                    bcsu(dcol_bf[:]), Alu.is_gt)
                dpc = cpool.tile([P, JL], dt.bfloat16)
                nc.vector.tensor_tensor(
                    dpc[:].rearrange(su, s=16, u=NT),
                    diagm[:].rearrange(su, s=16, u=NT),
                    bcsu(ce4[:, 0, :]), Alu.mult)
                dpe = cpool.tile([P, JL], dt.bfloat16)
                nc.vector.tensor_tensor(
                    dpe[:].rearrange(su, s=16, u=NT),
                    diagm[:].rearrange(su, s=16, u=NT),
                    bcsu(ce4[:, 1, :]), Alu.mult)
                # consume with (tile, slot) traversal into rows 32/33
                for c in range(2):
                    mv_c = dpc[:].rearrange("p (s u) -> p u s",
                                            s=16, u=NT)[:, 32 * c:32 * c + 32, :]
                    mv_e = dpe[:].rearrange("p (s u) -> p u s",
                                            s=16, u=NT)[:, 32 * c:32 * c + 32, :]
                    nc.tensor.matmul(ps[32:33, 512 * c:512 * c + 512],
                                     selc[:, 0:1], mv_c,
                                     start=False, stop=False,
                                     skip_group_check=True,
                                     tile_position=(0, 32))
                    nc.tensor.matmul(ps[32:34, 512 * c:512 * c + 512],
                                     selc[:, 1:3], mv_e,
                                     start=False, stop=False,
                                     skip_group_check=True,
                                     tile_position=(0, 32))

            # selc = [1, 0, 1]: col 0 -> row 32 (dpc), cols 1:3 -> row 33 (dpe)
            selc = cpool.tile([P, 3], dt.bfloat16)
            nc.gpsimd.memset(selc[:], 0.0)
            nc.gpsimd.memset(selc[:, 0:1], 1.0)
            nc.gpsimd.memset(selc[:, 2:3], 1.0)

            w64_ps = pspool.tile([NT, 1], dt.float32, name="w64ps")
            c64_ps = pspool.tile([NT, 1], dt.float32, name="c64ps")
            e64_ps = pspool.tile([NT, 1], dt.float32, name="e64ps")

            def emit_aggregates():
                # tile aggregates W64/C64/E64 [64,1] via data-as-stationary
                nc.tensor.matmul(w64_ps[:], expp[:], onesKf[:],
                                 start=True, stop=True, skip_group_check=True)
                nc.tensor.matmul(c64_ps[:], ccol[:], onesKf[:],
                                 start=True, stop=True, skip_group_check=True)
                nc.tensor.matmul(e64_ps[:], ecol[:], onesKb[:],
                                 start=True, stop=True, skip_group_check=True)
                # st64a = [0,0,+W64,128];  st64d = [+C64,+E64,-W64,-128]
                nc.vector.tensor_copy(st64a[:, 2:3], w64_ps[:])
                nc.vector.tensor_copy(st64d[:, 0:1], c64_ps[:])
                nc.vector.tensor_copy(st64d[:, 1:2], e64_ps[:])
                nc.vector.tensor_scalar(st64d[:, 2:3], w64_ps[:], -1.0, None,
                                        Alu.mult)

            pending = []
            for t in range(NT):
                ws, wd = _win(t)
                g = t % NG
                mask = mpool.tile([P, 512], dt.bfloat16, tag="mask",
                                  name="mask")
                if _is_act_tile(t):
                    nc.scalar.activation(
                        mask[:, 0:wd], dj_sb[:, ws:ws + wd], Act.Tanh,
                        bias=dbig[:, t:t + 1], scale=BIG)
                    stat = ceh4
                else:
                    nc.vector.tensor_scalar(
                        mask[:, 0:wd], dj_sb[:, ws:ws + wd],
                        dcol[:, t:t + 1], None, Alu.is_gt)
                    stat = ce4

                def mm(t=t, mask=mask, stat=stat, ws=ws, wd=wd, g=g):
                    nc.tensor.matmul(
                        ps[32 * g:32 * g + 4, ws:ws + wd], stat[:, :, t],
                        mask[:, 0:wd], start=False, stop=(t >= NT - NG),
                        skip_group_check=True, tile_position=(0, 32 * g))

                if t < 2:
                    pending.append(mm)   # ce4 written by staging at t==2
                else:
                    if t == 2:
                        emit_vstaging()
                        for f in pending:
                            f()
                        pending.clear()
                        emit_aggregates()
                    mm()
                if t == 30:
                    emit_diag()
                if t == 40:
                    # A-term + tanh-deficit injection (rows 32..35); moving
                    # is the [64,64] matrix chunk-expanded 16x via stride-0
                    for c in range(2):
                        mq = q64x[:, 32 * c:32 * c + 32].unsqueeze(2)                             .broadcast_to([NT, 32, 16])
                        mv = v64x[:, 32 * c:32 * c + 32].unsqueeze(2)                             .broadcast_to([NT, 32, 16])
                        nc.tensor.matmul(ps[32:36, 512 * c:512 * c + 512],
                                         st64a[:], mq, start=False,
                                         stop=False, skip_group_check=True,
                                         tile_position=(0, 32))
                        nc.tensor.matmul(ps[32:36, 512 * c:512 * c + 512],
                                         st64d[:], mv, start=False,
                                         stop=False, skip_group_check=True,
                                         tile_position=(0, 32))
                if t == 55:
                    # V touches absorbing fold-scatter DMA waits
                    scratch = cpool.tile([1, 4], dt.float32)
                    nc.vector.tensor_copy(scratch[0:1, 0:1],
                                          fold128[32:33, 0:1])
                    nc.vector.tensor_copy(scratch[0:1, 1:2],
                                          fold128[64:65, 0:1])
                    nc.vector.tensor_copy(scratch[0:1, 2:3],
                                          fold128[96:97, 0:1])

            # ---- fold epilogue: prod first (feeds the slow ACT accum),
            # prod2 second (V reduces it meanwhile)
            prod = cpool.tile([P, JL], dt.float32)
            nc.vector.tensor_mul(prod[:], ps[:, 0:JL], fold128[:])
            prod2 = cpool.tile([P, 512], dt.float32)
            nc.vector.tensor_mul(prod2[:], ps[:, JL:DW], fold128[:, 0:512])
            junk = cpool.tile([P, JL], dt.float32)
            red_a = cpool.tile([P, 1], dt.float32)
            nc.scalar.activation(junk[:], prod[:], Act.Copy,
                                 accum_out=red_a[:])
            red_b = cpool.tile([P, 1], dt.float32)
            nc.vector.tensor_reduce(red_b[:], prod2[:], AX.X, Alu.add)
            red = cpool.tile([P, 1], dt.float32)
            nc.vector.tensor_add(red[:], red_a[:], red_b[:])
            ps_f = pspool.tile([2, 1], dt.float32, name="psf")
            nc.tensor.matmul(ps_f[:], fsel[:], red[:],
                             start=True, stop=True, skip_group_check=True)
            psf_sb = cpool.tile([2, 1], dt.float32)
            nc.vector.tensor_copy(psf_sb[:], ps_f[:])
            nc.sync.dma_start(out_d[0:1, 0:2], psf_sb[0:2, 0:1])

    nc.finalize()
    return nc


def _get_program():
    global _cached
    if _cached is None:
        _cached = _build()
    return _cached


def _reduce_output(results):
    parts = np.stack([np.asarray(r["out"], dtype=np.float64).reshape(2)
                      for r in results])
    tot = parts.sum(axis=0)
    return np.float32(tot[0] / tot[1]).reshape(())


def _matrices():
    q = np.zeros((NT, NT), dtype=_BF16)
    v = np.zeros((NT, NT), dtype=_BF16)
    for t in range(NT):
        for u in range(NT):
            dd = (u - t) % NT
            if 1 <= dd <= 31 or (dd == 32 and t < 32):
                q[t, u] = 1.0
                if _is_act_tile(t):
                    v[t, u] = 0.5
    return q, v


def _shard_inputs(preds, targets):
    p = np.ascontiguousarray(np.asarray(preds, dtype=np.float32).reshape(-1))
    d = np.ascontiguousarray(np.asarray(targets[:, 0], dtype=np.float32))
    e = np.ascontiguousarray(np.asarray(targets[:, 1], dtype=np.float32))

    dbf = d.astype(_BF16)
    dbf32 = dbf.astype(np.float32)       # bf16-valued f32 (consistent ties)
    d_col = np.ascontiguousarray(dbf32.reshape(NT, P).T)
    p_col = np.ascontiguousarray(p.reshape(NT, P).T)
    e_col = np.ascontiguousarray(e.reshape(NT, P).T)

    q64x, v64x = _matrices()
    fsel = np.zeros((P, 2), dtype=np.float32)
    for g in range(NG):
        fsel[32 * g + 0, 0] = 1.0
        fsel[32 * g + 2, 0] = 1.0
        fsel[32 * g + 1, 1] = 1.0
        fsel[32 * g + 3, 1] = 1.0

    u_ = np.repeat(np.arange(NT), 16)
    s_ = np.tile(np.arange(16), NT)
    # (s, u) ordering for the diag batch
    s2_ = np.repeat(np.arange(16), NT)
    u2_ = np.tile(np.arange(NT), 16)

    in_maps = []
    for k in range(NCORES):
        jglob = 128 * u_ + 8 * s_ + k        # local (u,s) order
        jglob_su = 128 * u2_ + 8 * s2_ + k   # (s,u) order
        djl = dbf[jglob]
        djw = np.empty(DW, dtype=_BF16)
        djw[0:JL] = djl
        djw[JL:DW] = djl[0:DW - JL]
        pj = p[jglob]
        smalls = np.concatenate(
            [d_col, e_col, e[jglob].reshape(P, 8)], axis=1).astype(_BF16)
        smallf = np.concatenate(
            [p_col, pj.reshape(P, 8), d_col], axis=1).astype(np.float32)
        in_maps.append({
            "dj": np.ascontiguousarray(
                np.broadcast_to(djl[None, :], (P, JL))),
            "smalls": np.ascontiguousarray(smalls),
            "smallf": np.ascontiguousarray(smallf),
            "ej": np.ascontiguousarray(e[jglob].reshape(1, JL)),
            "q64x": q64x,
            "v64x": v64x,
            "fsel": fsel,
        })
    return in_maps


def _run(preds, targets, trace=False):
    from concourse import bass_utils

    nc = _get_program()
    in_maps = _shard_inputs(preds, targets)
    last_err = None
    for _attempt in range(3):
        try:
            res = bass_utils.run_bass_kernel_spmd(
                nc, in_maps, list(range(NCORES)), trace=trace)
            break
        except Exception as e:
            last_err = e
    else:
        raise last_err
    out = _reduce_output(res.results)
    return out, res


def kernel(preds, targets):
    out, _ = _run(preds, targets, trace=False)
    return out


def kernel_traced(preds, targets):
    return _run(preds, targets, trace=True)


# revision 3
# speedup vs baseline: 1.0370x; 1.0370x over previous
"""Trainium2 Bass kernel v2: ExponentialConcordanceLoss via antisymmetric
pair-orientation (tournament) halving.

loss = S / T,  S = sum_{a,b: d_a<d_b} e_a exp(p_b - p_a),  T = #pairs.

Pairs split by 128-row tile (64 tiles):
 - cross-tile pairs (t,u) in tournament R ((u-t) mod 64 in 1..31, or ==32
   with t<32): both orientations fold via
     m_ab c_a w_b + m_ba c_b w_a = c_b w_a + m_ab (c_a w_b - c_b w_a)
   so only canonical masks are generated (HALF the comparisons).  The
   closed term sum_R W_t C_u (and pair count) is injected into the PSUM
   accumulators by a matmul against a chunk-expanded tournament matrix
   Q64x [64 x 1024].
 - same-tile pairs: direct both-orientation masks, batched as one
   [128, 1024] compare in (slot, tile) column order so the 16x broadcast
   APs stay 2x-mode eligible; consumed by matmuls with (tile, slot)
   traversal back into the main accumulators.

Sharding: core k owns j = 128u + 8s + k (u=tile/chunk, s=slot), local
order jl = 16u + s.  Tile t's cross mask covers a 512 (496 for t>=32)
wide window at column 16(t+1) of the chunk-doubled dj row [1536]; PSUM
accumulates in doubled column space, the fold adds the aliases.

Masks: Vector tensor_scalar is_gt {0,1} and Scalar ACT Tanh(BIG*(dj-d_i))
in {-1,+1} (exactly saturated; 0 on bf16 ties).  Tanh tiles use 0.5x
stationaries; their per-window deficit is repaired by one matmul against
a host-built coverage matrix V64x [64 x 1024] (0.5 where tile t is
Tanh-assigned and covers jl's chunk).  A single ACT function set
(exp_and_others: Exp+Tanh+Copy) means exactly one table load.

Stationaries per tile: ce4[:, :, t] = [c_a, e_a, -w_a, -1] (M=4), fold
rows per group: [w_row, ones, c_row, e_row]; all small row vectors are
built in [128, 8] column layout and DMA-transposed into the fold rows.
Both comparison sides use bf16-rounded durations so m_ab + m_ba = 1
holds except exact ties (handled by the tanh-tie midpoint).
"""

import numpy as np
import ml_dtypes

N = 8192
NCORES = 8
P = 128
NT = 64          # i-tiles / j-chunks
JL = 1024        # j per core
DW = 1536        # doubled-window column space
NG = 4           # PE column groups
BIG = float(2 ** 30)

_BF16 = ml_dtypes.bfloat16

# ACT-engine (tanh) mask tiles; rest on Vector.
def _is_act_tile(t):
    return t % 3 == 1 or t == 32


def _win(t):
    return 16 * (t + 1), (512 if t < 32 else 496)


_cached = None


def _build():
    from concourse import bacc, tile, mybir

    dt = mybir.dt
    Alu = mybir.AluOpType
    Act = mybir.ActivationFunctionType
    AX = mybir.AxisListType

    nc = bacc.Bacc("TRN2", target_bir_lowering=False, debug=False,
                   num_devices=NCORES)

    dj_d = nc.dram_tensor("dj", [P, JL], dt.bfloat16, kind="ExternalInput").ap()
    djsu_d = nc.dram_tensor("djsu", [P, JL], dt.bfloat16, kind="ExternalInput").ap()
    smalls_d = nc.dram_tensor("smalls", [P, 2 * NT + 8], dt.bfloat16,
                              kind="ExternalInput").ap()
    smallf_d = nc.dram_tensor("smallf", [P, 2 * NT + 8], dt.float32,
                              kind="ExternalInput").ap()
    ej_d = nc.dram_tensor("ej", [1, JL], dt.float32, kind="ExternalInput").ap()
    q64x_d = nc.dram_tensor("q64x", [NT, NT], dt.bfloat16, kind="ExternalInput").ap()
    v64x_d = nc.dram_tensor("v64x", [NT, NT], dt.bfloat16, kind="ExternalInput").ap()
    fsel_d = nc.dram_tensor("fsel", [P, 2], dt.float32, kind="ExternalInput").ap()
    out_d = nc.dram_tensor("out", [1, 2], dt.float32, kind="ExternalOutput").ap()

    with tile.TileContext(nc) as tc:
        with (
            tc.tile_pool(name="cpool", bufs=1) as cpool,
            tc.tile_pool(name="mpool", bufs=24) as mpool,
            tc.tile_pool(name="pspool", bufs=1, space="PSUM") as pspool,
        ):
            # ---- input DMAs; small/critical tensors first per queue
            dj_sb = cpool.tile([P, DW], dt.bfloat16)
            smalls = cpool.tile([P, 2 * NT + 8], dt.bfloat16)
            dcol_bf = smalls[:, 0:NT]
            ecol = smalls[:, NT:2 * NT]
            e8 = smalls[:, 2 * NT:2 * NT + 8]
            smallf = cpool.tile([P, 2 * NT + 8], dt.float32)
            pcol = smallf[:, 0:NT]
            pj8 = smallf[:, NT:NT + 8]
            dcol = smallf[:, NT + 8:2 * NT + 8]
            dj_su = cpool.tile([P, JL], dt.bfloat16)
            q64x = cpool.tile([NT, NT], dt.bfloat16)
            v64x = cpool.tile([NT, NT], dt.bfloat16)
            fsel = cpool.tile([P, 2], dt.float32)
            # sync queue: dj first half (V mask gate), then doubling copy
            nc.sync.dma_start(dj_sb[:, 0:512], dj_d[:, 0:512])
            nc.sync.dma_start(dj_sb[:, JL:DW], dj_sb[:, 0:512])
            nc.sync.dma_start(fsel[:], fsel_d[:])
            # gpsimd queue: dj second half, diag dj, coverage matrix
            nc.gpsimd.dma_start(dj_sb[:, 512:JL], dj_d[:, 512:JL])
            nc.gpsimd.dma_start(dj_su[:], djsu_d[:])
            nc.gpsimd.dma_start(v64x[:], v64x_d[:])
            # scalar queue: the combined smalls first (S+V staging gate)
            nc.scalar.dma_start(smalls[:], smalls_d[:])
            nc.scalar.dma_start(smallf[:], smallf_d[:])
            nc.scalar.dma_start(q64x[:], q64x_d[:])

            # fold128 rows 32g+[0..3] = [w_row, 1, c_row, e_row]; built in
            # row4x [1, 4096] then partition-scattered (32-aligned dst).
            fold128 = cpool.tile([P, JL], dt.float32)
            nc.gpsimd.memset(fold128[:], 0.0)
            row4x = cpool.tile([1, 4 * JL], dt.float32)
            nc.gpsimd.memset(row4x[0:1, JL:2 * JL], 1.0)
            nc.sync.dma_start(row4x[0:1, 3 * JL:4 * JL], ej_d[:])

            zstat = cpool.tile([P, P], dt.bfloat16)
            nc.gpsimd.memset(zstat[:], 0.0)
            onesKf = cpool.tile([P, 1], dt.float32)
            nc.gpsimd.memset(onesKf[:], 1.0)
            onesKb = cpool.tile([P, 1], dt.bfloat16)
            nc.gpsimd.memset(onesKb[:], 1.0)
            # A-term and deficit stationaries [64, 4]
            st64a = cpool.tile([NT, 4], dt.bfloat16)
            nc.gpsimd.memset(st64a[:], 0.0)
            nc.gpsimd.memset(st64a[:, 3:4], float(P))
            st64d = cpool.tile([NT, 4], dt.bfloat16)
            nc.gpsimd.memset(st64d[:], 0.0)
            nc.gpsimd.memset(st64d[:, 3:4], -float(P))

            # ---- Scalar staging (one ACT table: Exp/Tanh/Copy)
            expnp = cpool.tile([P, NT], dt.float32)
            nc.scalar.activation(expnp[:], pcol[:], Act.Exp, scale=-1.0)
            expp = cpool.tile([P, NT], dt.float32)
            nc.scalar.activation(expp[:], pcol[:], Act.Exp)
            dbig = cpool.tile([P, NT], dt.float32)
            nc.scalar.activation(dbig[:], dcol_bf[:], Act.Copy, scale=-BIG)
            exp8 = cpool.tile([P, 8], dt.float32)
            nc.scalar.activation(exp8[:], pj8[:], Act.Exp)
            enp8 = cpool.tile([P, 8], dt.float32)
            nc.scalar.activation(enp8[:], pj8[:], Act.Exp, scale=-1.0)

            # ---- Vector staging tiles (ops emitted inside the mask loop
            # so Vector starts masks the moment dj/dcol land)
            ce4 = cpool.tile([P, 4, NT], dt.bfloat16)
            ccol = cpool.tile([P, NT], dt.float32)
            ceh4 = cpool.tile([P, 4, NT], dt.bfloat16)
            c8 = cpool.tile([P, 8], dt.float32)

            def emit_vstaging():
                nc.vector.tensor_mul(ccol[:], expnp[:], ecol[:])
                nc.vector.tensor_copy(ce4[:, 0, :], ccol[:])
                nc.vector.tensor_copy(ce4[:, 1, :], ecol[:])
                nc.vector.tensor_scalar(ce4[:, 2, :], expp[:], -1.0, None,
                                        Alu.mult)
                nc.vector.memset(ce4[:, 3, :], -1.0)
                nc.vector.tensor_scalar(
                    ceh4[:].rearrange("p a b -> p (a b)"),
                    ce4[:].rearrange("p a b -> p (a b)"), 0.5, None, Alu.mult)
                nc.vector.tensor_mul(c8[:], enp8[:], e8[:])
                # row transposes [128,8] -> [1,1024] (col = 8p + r)
                nc.sync.dma_start(
                    row4x[0:1, 0:JL].rearrange("o (p r) -> o p r", p=P, r=8),
                    exp8[:])
                nc.gpsimd.dma_start(
                    row4x[0:1, 2 * JL:3 * JL].rearrange("o (p r) -> o p r",
                                                        p=P, r=8),
                    c8[:])
                # fold row scatter
                for g in range(NG):
                    nc.sync.dma_start(fold128[32 * g:32 * g + 4, :], row4x[:])

            # ---- PSUM zero fill + accumulation group start
            ps = pspool.tile([P, DW], dt.float32, name="ps")
            for c in range(0, DW, 512):
                nc.tensor.matmul(ps[:, c:c + 512], zstat[:],
                                 dj_sb[:, c:c + 512],
                                 start=True, stop=False, skip_group_check=True)

            # ---- mask loop
            def emit_diag():
                # same-tile pairs in (slot, tile) order: broadcast APs keep
                # stride-1 last dim -> 2x DVE mode
                def bcsu(ap):
                    return ap.unsqueeze(1).broadcast_to([P, 16, NT])

                su = "p (s u) -> p s u"
                diagm = cpool.tile([P, JL], dt.bfloat16)
                nc.vector.tensor_tensor(
                    diagm[:].rearrange(su, s=16, u=NT),
                    dj_su[:].rearrange(su, s=16, u=NT),
                    bcsu(dcol_bf[:]), Alu.is_gt)
                dpc = cpool.tile([P, JL], dt.bfloat16)
                nc.vector.tensor_tensor(
                    dpc[:].rearrange(su, s=16, u=NT),
                    diagm[:].rearrange(su, s=16, u=NT),
                    bcsu(ce4[:, 0, :]), Alu.mult)
                dpe = cpool.tile([P, JL], dt.bfloat16)
                nc.vector.tensor_tensor(
                    dpe[:].rearrange(su, s=16, u=NT),
                    diagm[:].rearrange(su, s=16, u=NT),
                    bcsu(ce4[:, 1, :]), Alu.mult)
                # consume with (tile, slot) traversal into rows 32/33
                for c in range(2):
                    mv_c = dpc[:].rearrange("p (s u) -> p u s",
                                            s=16, u=NT)[:, 32 * c:32 * c + 32, :]
                    mv_e = dpe[:].rearrange("p (s u) -> p u s",
                                            s=16, u=NT)[:, 32 * c:32 * c + 32, :]
                    nc.tensor.matmul(ps[32:33, 512 * c:512 * c + 512],
                                     selc[:, 0:1], mv_c,
                                     start=False, stop=False,
                                     skip_group_check=True,
                                     tile_position=(0, 32))
                    nc.tensor.matmul(ps[32:34, 512 * c:512 * c + 512],
                                     selc[:, 1:3], mv_e,
                                     start=False, stop=False,
                                     skip_group_check=True,
                                     tile_position=(0, 32))

            # selc = [1, 0, 1]: col 0 -> row 32 (dpc), cols 1:3 -> row 33 (dpe)
            selc = cpool.tile([P, 3], dt.bfloat16)
            nc.gpsimd.memset(selc[:], 0.0)
            nc.gpsimd.memset(selc[:, 0:1], 1.0)
            nc.gpsimd.memset(selc[:, 2:3], 1.0)

            w64_ps = pspool.tile([NT, 1], dt.float32, name="w64ps")
            c64_ps = pspool.tile([NT, 1], dt.float32, name="c64ps")
            e64_ps = pspool.tile([NT, 1], dt.float32, name="e64ps")

            def emit_aggregates():
                # tile aggregates W64/C64/E64 [64,1] via data-as-stationary
                nc.tensor.matmul(w64_ps[:], expp[:], onesKf[:],
                                 start=True, stop=True, skip_group_check=True)
                nc.tensor.matmul(c64_ps[:], ccol[:], onesKf[:],
                                 start=True, stop=True, skip_group_check=True)
                nc.tensor.matmul(e64_ps[:], ecol[:], onesKb[:],
                                 start=True, stop=True, skip_group_check=True)
                # st64a = [0,0,+W64,128];  st64d = [+C64,+E64,-W64,-128]
                nc.vector.tensor_copy(st64a[:, 2:3], w64_ps[:])
                nc.vector.tensor_copy(st64d[:, 0:1], c64_ps[:])
                nc.vector.tensor_copy(st64d[:, 1:2], e64_ps[:])
                nc.vector.tensor_scalar(st64d[:, 2:3], w64_ps[:], -1.0, None,
                                        Alu.mult)

            pending = []
            for t in range(NT):
                ws, wd = _win(t)
                g = t % NG
                mask = mpool.tile([P, 512], dt.bfloat16, tag="mask",
                                  name="mask")
                if _is_act_tile(t):
                    nc.scalar.activation(
                        mask[:, 0:wd], dj_sb[:, ws:ws + wd], Act.Tanh,
                        bias=dbig[:, t:t + 1], scale=BIG)
                    stat = ceh4
                else:
                    nc.vector.tensor_scalar(
                        mask[:, 0:wd], dj_sb[:, ws:ws + wd],
                        dcol[:, t:t + 1], None, Alu.is_gt)
                    stat = ce4

                def mm(t=t, mask=mask, stat=stat, ws=ws, wd=wd, g=g):
                    nc.tensor.matmul(
                        ps[32 * g:32 * g + 4, ws:ws + wd], stat[:, :, t],
                        mask[:, 0:wd], start=False, stop=(t >= NT - NG),
                        skip_group_check=True, tile_position=(0, 32 * g))

                if t < 2:
                    pending.append(mm)   # ce4 written by staging at t==2
                else:
                    if t == 2:
                        emit_vstaging()
                        for f in pending:
                            f()
                        pending.clear()
                        emit_aggregates()
                    mm()
                if t == 30:
                    emit_diag()
                if t == 40:
                    # A-term + tanh-deficit injection (rows 32..35); moving
                    # is the [64,64] matrix chunk-expanded 16x via stride-0
                    for c in range(2):
                        mq = q64x[:, 32 * c:32 * c + 32].unsqueeze(2)                             .broadcast_to([NT, 32, 16])
                        mv = v64x[:, 32 * c:32 * c + 32].unsqueeze(2)                             .broadcast_to([NT, 32, 16])
                        nc.tensor.matmul(ps[32:36, 512 * c:512 * c + 512],
                                         st64a[:], mq, start=False,
                                         stop=False, skip_group_check=True,
                                         tile_position=(0, 32))
                        nc.tensor.matmul(ps[32:36, 512 * c:512 * c + 512],
                                         st64d[:], mv, start=False,
                                         stop=False, skip_group_check=True,
                                         tile_position=(0, 32))
                if t == 55:
                    # V touches absorbing fold-scatter DMA waits
                    scratch = cpool.tile([1, 4], dt.float32)
                    nc.vector.tensor_copy(scratch[0:1, 0:1],
                                          fold128[32:33, 0:1])
                    nc.vector.tensor_copy(scratch[0:1, 1:2],
                                          fold128[64:65, 0:1])
                    nc.vector.tensor_copy(scratch[0:1, 2:3],
                                          fold128[96:97, 0:1])

            # ---- fold epilogue: prod first (feeds the slow ACT accum),
            # prod2 second (V reduces it meanwhile)
            prod = cpool.tile([P, JL], dt.float32)
            nc.vector.tensor_mul(prod[:], ps[:, 0:JL], fold128[:])
            prod2 = cpool.tile([P, 512], dt.float32)
            nc.vector.tensor_mul(prod2[:], ps[:, JL:DW], fold128[:, 0:512])
            junk = cpool.tile([P, JL], dt.float32)
            red_a = cpool.tile([P, 1], dt.float32)
            nc.scalar.activation(junk[:], prod[:], Act.Copy,
                                 accum_out=red_a[:])
            red_b = cpool.tile([P, 1], dt.float32)
            nc.vector.tensor_reduce(red_b[:], prod2[:], AX.X, Alu.add)
            red = cpool.tile([P, 1], dt.float32)
            nc.vector.tensor_add(red[:], red_a[:], red_b[:])
            ps_f = pspool.tile([2, 1], dt.float32, name="psf")
            nc.tensor.matmul(ps_f[:], fsel[:], red[:],
                             start=True, stop=True, skip_group_check=True)
            psf_sb = cpool.tile([2, 1], dt.float32)
            nc.vector.tensor_copy(psf_sb[:], ps_f[:])
            nc.sync.dma_start(out_d[0:1, 0:2], psf_sb[0:2, 0:1])

    nc.finalize()
    return nc


def _get_program():
    global _cached
    if _cached is None:
        _cached = _build()
    return _cached


def _reduce_output(results):
    parts = np.stack([np.asarray(r["out"], dtype=np.float64).reshape(2)
                      for r in results])
    tot = parts.sum(axis=0)
    return np.float32(tot[0] / tot[1]).reshape(())


def _matrices():
    q = np.zeros((NT, NT), dtype=_BF16)
    v = np.zeros((NT, NT), dtype=_BF16)
    for t in range(NT):
        for u in range(NT):
            dd = (u - t) % NT
            if 1 <= dd <= 31 or (dd == 32 and t < 32):
                q[t, u] = 1.0
                if _is_act_tile(t):
                    v[t, u] = 0.5
    return q, v


def _shard_inputs(preds, targets):
    p = np.ascontiguousarray(np.asarray(preds, dtype=np.float32).reshape(-1))
    d = np.ascontiguousarray(np.asarray(targets[:, 0], dtype=np.float32))
    e = np.ascontiguousarray(np.asarray(targets[:, 1], dtype=np.float32))

    dbf = d.astype(_BF16)
    dbf32 = dbf.astype(np.float32)       # bf16-valued f32 (consistent ties)
    d_col = np.ascontiguousarray(dbf32.reshape(NT, P).T)
    p_col = np.ascontiguousarray(p.reshape(NT, P).T)
    e_col = np.ascontiguousarray(e.reshape(NT, P).T)

    q64x, v64x = _matrices()
    fsel = np.zeros((P, 2), dtype=np.float32)
    for g in range(NG):
        fsel[32 * g + 0, 0] = 1.0
        fsel[32 * g + 2, 0] = 1.0
        fsel[32 * g + 1, 1] = 1.0
        fsel[32 * g + 3, 1] = 1.0

    u_ = np.repeat(np.arange(NT), 16)
    s_ = np.tile(np.arange(16), NT)
    # (s, u) ordering for the diag batch
    s2_ = np.repeat(np.arange(16), NT)
    u2_ = np.tile(np.arange(NT), 16)

    in_maps = []
    for k in range(NCORES):
        jglob = 128 * u_ + 8 * s_ + k        # local (u,s) order
        jglob_su = 128 * u2_ + 8 * s2_ + k   # (s,u) order
        djl = dbf[jglob]
        djw = np.empty(DW, dtype=_BF16)
        djw[0:JL] = djl
        djw[JL:DW] = djl[0:DW - JL]
        pj = p[jglob]
        smalls = np.concatenate(
            [d_col, e_col, e[jglob].reshape(P, 8)], axis=1).astype(_BF16)
        smallf = np.concatenate(
            [p_col, pj.reshape(P, 8), d_col], axis=1).astype(np.float32)
        in_maps.append({
            "dj": np.ascontiguousarray(
                np.broadcast_to(djl[None, :], (P, JL))),
            "djsu": np.ascontiguousarray(
                np.broadcast_to(dbf[jglob_su][None, :], (P, JL))),
            "smalls": np.ascontiguousarray(smalls),
            "smallf": np.ascontiguousarray(smallf),
            "ej": np.ascontiguousarray(e[jglob].reshape(1, JL)),
            "q64x": q64x,
            "v64x": v64x,
            "fsel": fsel,
        })
    return in_maps


def _run(preds, targets, trace=False):
    from concourse import bass_utils

    nc = _get_program()
    in_maps = _shard_inputs(preds, targets)
    last_err = None
    for _attempt in range(3):
        try:
            res = bass_utils.run_bass_kernel_spmd(
                nc, in_maps, list(range(NCORES)), trace=trace)
            break
        except Exception as e:
            last_err = e
    else:
        raise last_err
    out = _reduce_output(res.results)
    return out, res


def kernel(preds, targets):
    out, _ = _run(preds, targets, trace=False)
    return out


def kernel_traced(preds, targets):
    return _run(preds, targets, trace=True)
